# revision 87
# baseline (speedup 1.0000x reference)
import os
import numpy as np
import ml_dtypes

import concourse.bass as bass
import concourse.tile as tile
from concourse import bacc, mybir
from concourse.bass import ts
from concourse.bass_utils import run_bass_kernel_spmd
from concourse.masks import make_identity

L, B, Q, D, NC, CS = 6, 32, 900, 256, 10, 10
EPS = 1e-5
NCORES = 8
BPC = B // NCORES          # 4 samples per core
T = BPC * Q                # 3600 tokens per core
NT = 29                    # token tiles of 128
TP = NT * 128              # 3712 padded tokens
BF16 = mybir.dt.bfloat16
F32 = mybir.dt.float32
AF = mybir.ActivationFunctionType
ALU = mybir.AluOpType

_cache = {}

# chunking: groups of token tiles processed per pipeline step
CHUNKS = [(0, 3), (3, 3), (6, 4), (10, 4), (14, 4), (18, 4), (22, 4), (26, 3)]


QUAKE = 0x5F3759DF


def _build_fast():
    """Fast path: assumes zero biases on the hidden linears/LNs (gains may be
    arbitrary nonzero; they are folded into the weights host-side along with
    LN mean-centering).  LN normalization then reduces to one per-token
    scalar applied at the cls head (computed as rsqrt of a PE-reduced sum of
    squares, via Quake seed + 2 Newton iterations on DVE)."""
    nc = bacc.Bacc("TRN2", target_bir_lowering=False, debug=False,
                   enable_asserts=False, num_devices=NCORES)
    hsT = nc.dram_tensor("hsT", [L, 128, 2, TP], BF16, kind="ExternalInput").ap()
    wmm = nc.dram_tensor("wmm", [L, 128, 16, 128], BF16, kind="ExternalInput").ap()
    w3s = nc.dram_tensor("w3s", [L, 128, 4, 10], BF16, kind="ExternalInput").ap()
    svec = nc.dram_tensor("svec", [L, 128, 2], BF16, kind="ExternalInput").ap()
    b3rep = nc.dram_tensor("b3rep", [L, 128, 10], F32, kind="ExternalInput").ap()
    rb3rep = nc.dram_tensor("rb3rep", [L, 128, 10], F32, kind="ExternalInput").ap()
    cepsr = nc.dram_tensor("cepsr", [L, 128, 1], F32, kind="ExternalInput").ap()
    ivs = nc.dram_tensor("ivs", [L, 128, NT, 3], F32, kind="ExternalInput").ap()
    o_cls = nc.dram_tensor("o_cls", [L, 128, NT, 10], F32, kind="ExternalOutput").ap()
    o_crd = nc.dram_tensor("o_crd", [L, 128, NT, 10], F32, kind="ExternalOutput").ap()
    I32 = mybir.dt.int32

    with tile.TileContext(nc) as tc:
        with (
            tc.tile_pool(name="const", bufs=1) as cp,
            tc.tile_pool(name="io", bufs=2) as iop,
            tc.tile_pool(name="act", bufs=1) as ap_,
            tc.tile_pool(name="sm", bufs=2) as smp,
            tc.tile_pool(name="ps", bufs=4, space="PSUM") as pp,
        ):
            qk = cp.tile([128, 1], I32)
            nc.vector.memset(qk[:], QUAKE)

            def load_layer(l):
                w_t = iop.tile([128, 16, 128], BF16, tag="w", name=f"w{l}")
                hs_t = iop.tile([128, 2, TP], BF16, tag="hs", name=f"hs{l}")
                nc.sync.dma_start(w_t[:, 0:4, :], wmm[l][:, 0:4, :])
                nc.sync.dma_start(hs_t[:, :, 0:384], hsT[l][:, :, 0:384])
                nc.sync.dma_start(w_t[:, 4:16, :], wmm[l][:, 4:16, :])
                for r0, rw in ((384, 1664), (2048, 1664)):
                    nc.sync.dma_start(hs_t[:, :, r0:r0 + rw],
                                      hsT[l][:, :, r0:r0 + rw])
                w3_t = iop.tile([128, 4, 10], BF16, tag="w3", name=f"w3{l}")
                nc.sync.dma_start(w3_t[:], w3s[l])
                sv_t = iop.tile([128, 2], BF16, tag="sv", name=f"sv{l}")
                nc.sync.dma_start(sv_t[:], svec[l])
                b3_t = iop.tile([128, 10], F32, tag="b3", name=f"b3{l}")
                nc.sync.dma_start(b3_t[:], b3rep[l])
                rb_t = iop.tile([128, 10], F32, tag="rb", name=f"rb{l}")
                nc.sync.dma_start(rb_t[:], rb3rep[l])
                ce_t = iop.tile([128, 1], F32, tag="ce", name=f"ce{l}")
                nc.sync.dma_start(ce_t[:], cepsr[l])
                iv_t = iop.tile([128, NT, 3], F32, tag="iv", name=f"iv{l}")
                nc.sync.dma_start(iv_t[:], ivs[l])
                return dict(hs=hs_t, w=w_t, w3=w3_t, sv=sv_t, b3=b3_t,
                            rb=rb_t, ce=ce_t, iv=iv_t)

            lay = {0: load_layer(0)}
            pending_fin = None
            pending_heads = None

            for l in range(L):
                if l + 1 < L:
                    lay[l + 1] = load_layer(l + 1)
                tl = lay.pop(l)
                hs_t, w_t = tl["hs"], tl["w"]
                # per-layer activation tensors [128, 2(kh), TP] bf16
                a1 = ap_.tile([128, 2, TP], BF16, tag="a1", name=f"a1_{l}", bufs=2)
                y1 = ap_.tile([128, 2, TP], BF16, tag="y1", name=f"y1_{l}", bufs=2)
                v2b = ap_.tile([128, 2, TP], BF16, tag="v2b", name=f"v2b_{l}",
                               bufs=2)
                a2 = ap_.tile([128, 2, TP], BF16, tag="a2", name=f"a2_{l}")
                sq2 = ap_.tile([128, 2, TP], BF16, tag="sq2", name=f"sq2_{l}")
                y2 = ap_.tile([128, 2, TP], BF16, tag="y2", name=f"y2_{l}")
                # heads staging: [:, :, 0:10]=cls raw, 10:20=reg tmp, 20=m
                hds = smp.tile([128, NT, 21], F32, tag="hds", name=f"hds_{l}")
                cls_sb = smp.tile([128, NT, 10], F32, tag="cls", name=f"cls_{l}")
                tmp_sb = hds[:, :, 10:20]
                m_sb = smp.tile([128, NT], F32, tag="m", name=f"m_{l}")

                def linear(i, src, c0, cw, thunks=None):
                    """z[:, mh, cols] = sum_kh W[i,kh,mh]^T @ src[:, kh, cols]
                    Interleaves up to 4 head-matmul thunks after each big
                    matmul so their dispatch hides under the PE streams."""
                    z = pp.tile([128, 2, 512], F32, tag="mm", name=f"z{i}")
                    for mh in range(2):
                        for kh in range(2):
                            nc.tensor.matmul(
                                z[:, mh, 0:cw], w_t[:, i * 4 + kh * 2 + mh, :],
                                src[:, kh, c0:c0 + cw],
                                start=(kh == 0), stop=(kh == 1))
                            if thunks is not None:
                                for _ in range(4):
                                    th = next(thunks, None)
                                    if th is None:
                                        break
                                    th()
                    return z

                def heads_thunks(c, tl=tl, a2=a2, sq2=sq2, y2=y2, hds=hds):
                    """Head matmuls for chunk c as a list of emission thunks.
                    The whole hd tile is ONE psum accumulation group: start
                    zeroes the full 2KB zero-region; later matmuls accumulate
                    into disjoint column subranges; order start-first,
                    stop-last must be preserved."""
                    t0, tn = CHUNKS[c]
                    hd = pp.tile([128, 4, 21], F32, tag="mm", name="hd",
                                 padded_shape=[128, 4, 128])
                    th = []
                    for j in range(tn):
                        tt = t0 + j
                        for kh in range(2):
                            th.append(lambda j=j, kh=kh, tt=tt: nc.tensor.matmul(
                                hd[:, j, 0:10], a2[:, kh, ts(tt, 128)],
                                tl["w3"][:, kh, :],
                                start=(j == 0 and kh == 0), stop=False))
                            th.append(lambda j=j, kh=kh, tt=tt: nc.tensor.matmul(
                                hd[:, j, 20:21], sq2[:, kh, ts(tt, 128)],
                                tl["sv"][:, kh:kh + 1],
                                start=False, stop=False))
                    for j in range(tn):
                        tt = t0 + j
                        for kh in range(2):
                            th.append(lambda j=j, kh=kh, tt=tt: nc.tensor.matmul(
                                hd[:, j, 10:20], y2[:, kh, ts(tt, 128)],
                                tl["w3"][:, 2 + kh, :],
                                start=False,
                                stop=(j == tn - 1 and kh == 1)))
                    sl = slice(t0, t0 + tn)
                    th.append(lambda: nc.vector.tensor_copy(hds[:, sl, :],
                                                            hd[:, 0:tn, :]))
                    return th[:2 * 2 * tn], th[2 * 2 * tn:]

                yq = smp.tile([128, NT], F32, tag="yq", name="yq")
                t1_ = smp.tile([128, NT], F32, tag="t1", name="t1")
                s_sb = smp.tile([128, NT], F32, tag="s", name=f"s_{l}")
                su = smp.tile([128, NT, 3], F32, tag="su", name="su")
                sg = smp.tile([128, NT, 3], F32, tag="sg", name="sg")
                crd = smp.tile([128, NT, 10], F32, tag="crd", name="crd")

                def finish(a, b, iters=2, l=l, tl=tl, hds=hds, tmp_sb=tmp_sb,
                           m_sb=m_sb, cls_sb=cls_sb, yq=yq, t1_=t1_,
                           s_sb=s_sb, su=su, sg=sg, crd=crd):
                    """Tail work (rsqrt, cls scale, coords, DMA) for tile
                    range [a, b) — heads for those tiles must be drained."""
                    w = b - a
                    tsl = slice(a, b)
                    # rsqrt(m) via Quake seed + 2 Newton iterations (DVE)
                    nc.vector.tensor_scalar_add(m_sb[:, tsl],
                                                hds[:, tsl, 20], tl["ce"][:])
                    mi = m_sb[:, tsl].bitcast(I32)
                    yqi = yq[:, tsl].bitcast(I32)
                    nc.vector.tensor_scalar(yqi, mi, 1, None,
                                            ALU.logical_shift_right)
                    qkw = qk[:].broadcast_to((128, w))
                    nc.vector.tensor_tensor(yqi, qkw, yqi, ALU.subtract)
                    for it in range(iters):
                        src = yq if it == 0 else s_sb
                        nc.vector.tensor_tensor(t1_[:, tsl], src[:, tsl],
                                                src[:, tsl], ALU.mult)
                        nc.vector.tensor_tensor(t1_[:, tsl], t1_[:, tsl],
                                                m_sb[:, tsl], ALU.mult)
                        nc.vector.tensor_scalar(t1_[:, tsl], t1_[:, tsl],
                                                -0.5, 1.5, ALU.mult, ALU.add)
                        nc.vector.tensor_tensor(s_sb[:, tsl], src[:, tsl],
                                                t1_[:, tsl], ALU.mult)
                    # cls = clsr * s + b3
                    svw = s_sb[:, tsl].unsqueeze(2).broadcast_to((128, w, 10))
                    nc.gpsimd.tensor_tensor(cls_sb[:, tsl, :],
                                            hds[:, tsl, 0:10], svw, ALU.mult)
                    b3w = tl["b3"].unsqueeze(1).broadcast_to((128, w, 10))
                    nc.gpsimd.tensor_tensor(cls_sb[:, tsl, :],
                                            cls_sb[:, tsl, :], b3w, ALU.add)
                    # coord postprocess: sigmoid(tmp + invsig(ref))
                    nc.gpsimd.tensor_tensor(su[:, tsl, 0:2],
                                            tmp_sb[:, tsl, 0:2],
                                            tl["iv"][:, tsl, 0:2], ALU.add)
                    nc.gpsimd.tensor_tensor(su[:, tsl, 2:3],
                                            tmp_sb[:, tsl, 4:5],
                                            tl["iv"][:, tsl, 2:3], ALU.add)
                    nc.scalar.activation(sg[:, tsl, :], su[:, tsl, :],
                                         AF.Sigmoid)
                    nc.gpsimd.tensor_scalar(crd[:, tsl, 0:2], sg[:, tsl, 0:2],
                                            102.4, -51.2, ALU.mult, ALU.add)
                    nc.gpsimd.tensor_scalar(crd[:, tsl, 4:5], sg[:, tsl, 2:3],
                                            8.0, -5.0, ALU.mult, ALU.add)
                    rbw1 = tl["rb"][:, 2:4].unsqueeze(1).broadcast_to(
                        (128, w, 2))
                    nc.gpsimd.tensor_tensor(crd[:, tsl, 2:4],
                                            tmp_sb[:, tsl, 2:4], rbw1, ALU.add)
                    rbw2 = tl["rb"][:, 5:10].unsqueeze(1).broadcast_to(
                        (128, w, 5))
                    nc.gpsimd.tensor_tensor(crd[:, tsl, 5:10],
                                            tmp_sb[:, tsl, 5:10], rbw2,
                                            ALU.add)
                    nc.sync.dma_start(o_cls[l][:, tsl], cls_sb[:, tsl, :])
                    nc.sync.dma_start(o_crd[l][:, tsl], crd[:, tsl, :])

                for c, (t0, tn) in enumerate(CHUNKS):
                    c0, cw = t0 * 128, tn * 128
                    cs = slice(c0, c0 + cw)
                    z1 = linear(0, hs_t, c0, cw)
                    nc.scalar.activation(a1[:, :, cs], z1[:, :, 0:cw], AF.Relu)
                    r1 = linear(1, hs_t, c0, cw)
                    nc.vector.tensor_scalar_max(y1[:, :, cs], r1[:, :, 0:cw],
                                                0.0)
                    if c > 0:
                        ta, tb = heads_thunks(c - 1)
                        for th in list(ta) + list(tb):
                            th()
                    elif pending_heads is not None:
                        ta, tb = pending_heads
                        for th in list(ta) + list(tb):
                            th()
                        pending_heads = None
                    if c == 2 and pending_fin is not None:
                        pending_fin(0, NT)
                        pending_fin = None
                    if c == len(CHUNKS) - 1 and l == L - 1:
                        finish(0, CHUNKS[c - 1][0] + CHUNKS[c - 1][1])
                    z2 = linear(2, a1, c0, cw)
                    nc.scalar.copy(v2b[:, :, cs], z2[:, :, 0:cw])
                    nc.gpsimd.tensor_scalar_max(a2[:, :, cs], v2b[:, :, cs],
                                                0.0)
                    nc.vector.tensor_tensor(sq2[:, :, cs], v2b[:, :, cs],
                                            v2b[:, :, cs], ALU.mult)
                    r2 = linear(3, y1, c0, cw)
                    nc.scalar.activation(y2[:, 0, cs], r2[:, 0, 0:cw], AF.Relu)
                    nc.vector.tensor_scalar_max(y2[:, 1, cs], r2[:, 1, 0:cw],
                                                0.0)
                last = len(CHUNKS) - 1
                if l == L - 1:
                    ta, tb = heads_thunks(last)
                    for th in list(ta) + list(tb):
                        th()
                    finish(CHUNKS[last - 1][0] + CHUNKS[last - 1][1], NT)
                else:
                    pending_heads = heads_thunks(last)
                    pending_fin = finish

    nc.compile()
    return nc


def _prep_core_fast(c, hs, init_reference, inter_references, W, rb3):
    bs = slice(c * BPC, (c + 1) * BPC)
    h = hs[:, :, bs, :]                                   # [L,Q,4,D]
    hsT = np.zeros((L, D, TP), np.float32)
    hsT[:, :, :T] = h.transpose(0, 3, 2, 1).reshape(L, D, BPC * Q)
    hsT = (hsT.reshape(L, 2, 128, TP).transpose(0, 2, 1, 3)
           .astype(ml_dtypes.bfloat16))
    hsT = np.ascontiguousarray(hsT)

    refs = np.concatenate([init_reference[None], inter_references[:L - 1]], 0)
    r = np.clip(refs[:, bs].reshape(L, T, 3), 0.0, 1.0)   # [L,3600,3]
    iv = np.zeros((L, TP, 3), np.float32)
    iv[:, :T] = np.log(np.maximum(r, EPS) / np.maximum(1.0 - r, EPS))
    iv[:, :T] += rb3[:, None, [0, 1, 4]]   # fold reg_b3 into sigmoid offsets
    ivs = np.ascontiguousarray(
        iv.reshape(L, NT, 128, 3).transpose(0, 2, 1, 3))
    return dict(hsT=hsT, ivs=ivs, **W)


def _host_weights_fast(cls_w1, ln1_g, cls_w2, ln2_g, cls_w3, cls_b3,
                       reg_w1, reg_w2, reg_w3, reg_b3):
    g1 = np.asarray(ln1_g, np.float32).reshape(L, D)
    g2 = np.asarray(ln2_g, np.float32).reshape(L, D)
    W1 = np.asarray(cls_w1, np.float32)
    W2 = np.asarray(cls_w2, np.float32)
    W1p = (W1 - W1.mean(axis=2, keepdims=True)) * g1[:, None, :]
    W2p = (W2 - W2.mean(axis=2, keepdims=True)) * g2[:, None, :]
    R1 = np.asarray(reg_w1, np.float32)
    R2 = np.asarray(reg_w2, np.float32)

    wmm = np.zeros((L, 128, 16, 128), np.float32)
    for i, Wt in enumerate((W1p, R1, W2p, R2)):
        for kh in range(2):
            for mh in range(2):
                blk = Wt[:, kh * 128:(kh + 1) * 128, mh * 128:(mh + 1) * 128]
                wmm[:, :, i * 4 + kh * 2 + mh, :] = blk
    wmm = wmm.astype(ml_dtypes.bfloat16)

    W3c = np.asarray(cls_w3, np.float32)
    W3r = np.asarray(reg_w3, np.float32)
    w3s = np.zeros((L, 128, 4, 10), np.float32)
    for kh in range(2):
        w3s[:, :, kh, :] = W3c[:, kh * 128:(kh + 1) * 128, :]
        w3s[:, :, 2 + kh, :] = W3r[:, kh * 128:(kh + 1) * 128, :]
    w3s = w3s.astype(ml_dtypes.bfloat16)

    sv = (1.0 / (D * g2 * g2)).reshape(L, 2, 128).transpose(0, 2, 1)
    sv = np.ascontiguousarray(sv).astype(ml_dtypes.bfloat16)

    b3rep = np.broadcast_to(
        np.asarray(cls_b3, np.float32).reshape(L, 1, NC), (L, 128, NC))
    b3rep = np.ascontiguousarray(b3rep)
    rb3rep = np.broadcast_to(
        np.asarray(reg_b3, np.float32).reshape(L, 1, CS), (L, 128, CS))
    rb3rep = np.ascontiguousarray(rb3rep)

    # ceps[l] = EPS*(var1_est + EPS); var1_est = mean_j sum_i W1p^2 (h~N(0,1))
    var1_est = (W1p ** 2).sum(axis=1).mean(axis=1)           # [L]
    ceps = (EPS * (var1_est + EPS)).astype(np.float32)
    cepsr = np.ascontiguousarray(
        np.broadcast_to(ceps[:, None, None], (L, 128, 1)).astype(np.float32))
    return dict(wmm=wmm, w3s=w3s, svec=sv, b3rep=b3rep, rb3rep=rb3rep,
                cepsr=cepsr)


def _fast_ok(cls_b1, cls_b2, ln1_b, ln2_b, ln1_g, ln2_g, reg_b1, reg_b2):
    for b in (cls_b1, cls_b2, ln1_b, ln2_b, reg_b1, reg_b2):
        if np.abs(np.asarray(b)).max() > 0:
            return False
    for g in (ln1_g, ln2_g):
        if np.abs(np.asarray(g)).min() < 1e-3:
            return False
    return True


def kernel(hs, init_reference, inter_references,
           cls_w1, cls_b1, ln1_g, ln1_b, cls_w2, cls_b2, ln2_g, ln2_b,
           cls_w3, cls_b3, reg_w1, reg_b1, reg_w2, reg_b2, reg_w3, reg_b3):
    hs = np.asarray(hs, np.float32)
    init_reference = np.asarray(init_reference, np.float32)
    inter_references = np.asarray(inter_references, np.float32)

    if _fast_ok(cls_b1, cls_b2, ln1_b, ln2_b, ln1_g, ln2_g, reg_b1, reg_b2):
        W = _host_weights_fast(cls_w1, ln1_g, cls_w2, ln2_g, cls_w3, cls_b3,
                               reg_w1, reg_w2, reg_w3, reg_b3)
        if "ncf" not in _cache:
            _cache["ncf"] = _build_fast()
        nc = _cache["ncf"]
        rb3 = np.asarray(reg_b3, np.float32).reshape(L, CS)
        in_maps = [_prep_core_fast(c, hs, init_reference, inter_references,
                                   W, rb3)
                   for c in range(NCORES)]
        res = run_bass_kernel_spmd(nc, in_maps, core_ids=list(range(NCORES)),
                                   trace=bool(os.environ.get("KTRACE")))
        _cache["last_result"] = res
        out = np.zeros((2, L, B, Q, 10), np.float32)
        for c in range(NCORES):
            for j, k in enumerate(("o_cls", "o_crd")):
                v = res.results[c][k]        # [L,128,NT,10]
                v = v.transpose(0, 2, 1, 3).reshape(L, TP, 10)[:, :T]
                out[j, :, c * BPC:(c + 1) * BPC] = v.reshape(L, BPC, Q, 10)
        return out

    return _kernel_general(hs, init_reference, inter_references,
                           cls_w1, cls_b1, ln1_g, ln1_b, cls_w2, cls_b2,
                           ln2_g, ln2_b, cls_w3, cls_b3, reg_w1, reg_b1,
                           reg_w2, reg_b2, reg_w3, reg_b3)


# ======================================================================
# General fallback path (arbitrary biases/gains) — original implementation
# ======================================================================

def _build_general():
    nc = bacc.Bacc("TRN2", target_bir_lowering=False, debug=False,
                   enable_asserts=False, num_devices=NCORES)
    hsT = nc.dram_tensor("hsT", [L, 2, 128, TP], BF16, kind="ExternalInput").ap()
    wts = nc.dram_tensor("wts", [L, 4, 2, 128, 256], BF16, kind="ExternalInput").ap()
    w3p = nc.dram_tensor("w3p", [L, 2, 2, 128, 10], BF16, kind="ExternalInput").ap()
    brow = nc.dram_tensor("brow", [L, 4, 1, 256], BF16, kind="ExternalInput").ap()
    scal = nc.dram_tensor("scal", [L, 6, 256, 1], F32, kind="ExternalInput").ap()
    Rh = nc.dram_tensor("Rh", [L, 128, NT, 5], F32, kind="ExternalInput").ap()
    Bh = nc.dram_tensor("Bh", [L, 128, NT, 5], F32, kind="ExternalInput").ap()
    o_cls = nc.dram_tensor("o_cls", [L, NT, 10, 128], F32, kind="ExternalOutput").ap()
    o_crd = nc.dram_tensor("o_crd", [L, NT, 10, 128], F32, kind="ExternalOutput").ap()

    with tile.TileContext(nc) as tc:
        with (
            tc.tile_pool(name="const", bufs=1) as cp,
            tc.tile_pool(name="wk", bufs=4) as wk,
            tc.tile_pool(name="st", bufs=8) as stp,
            tc.tile_pool(name="acc", bufs=2) as accp,
            tc.tile_pool(name="ps", bufs=4, space="PSUM") as pp,
        ):
            ident = cp.tile([128, 128], BF16)
            make_identity(nc, ident[:])
            ones = cp.tile([1, 128], BF16)
            nc.vector.memset(ones[:], 1.0)
            eps_t = cp.tile([128, 1], F32)
            nc.vector.memset(eps_t[:], EPS)
            zer_t = cp.tile([128, 1], F32)
            nc.vector.memset(zer_t[:], 0.0)

            hs_sb, w_sb, w3_sb, br_sb, sc_sb, R_sb, Bm_sb = [], [], [], [], [], [], []
            for l in range(L):
                hl = [cp.tile([128, TP], BF16, tag=f"hs{l}{k}", name=f"hs{l}{k}") for k in range(2)]
                for k in range(2):
                    nc.sync.dma_start(hl[k][:], hsT[l, k])
                hs_sb.append(hl)
                wl = [[cp.tile([128, 256], BF16, tag=f"w{l}{i}{k}", name=f"w{l}{i}{k}") for k in range(2)]
                      for i in range(4)]
                for i in range(4):
                    for k in range(2):
                        nc.sync.dma_start(wl[i][k][:], wts[l, i, k])
                w_sb.append(wl)
                w3l = [[cp.tile([128, 10], BF16, tag=f"w3{l}{i}{k}", name=f"w3{l}{i}{k}") for k in range(2)]
                       for i in range(2)]
                for i in range(2):
                    for k in range(2):
                        nc.sync.dma_start(w3l[i][k][:], w3p[l, i, k])
                w3_sb.append(w3l)
                brl = [cp.tile([1, 256], BF16, tag=f"br{l}{i}", name=f"br{l}{i}") for i in range(4)]
                for i in range(4):
                    nc.sync.dma_start(brl[i][:], brow[l, i])
                br_sb.append(brl)
                scl = [[cp.tile([128, 1], F32, tag=f"sc{l}{i}{k}", name=f"sc{l}{i}{k}") for k in range(2)]
                       for i in range(6)]
                for i in range(6):
                    for k in range(2):
                        nc.sync.dma_start(scl[i][k][:], scal[l, i, ts(k, 128)])
                sc_sb.append(scl)
                rt = cp.tile([128, NT, 5], F32, tag=f"R{l}", name=f"Rt{l}")
                bt = cp.tile([128, NT, 5], F32, tag=f"B{l}", name=f"Bt{l}")
                nc.sync.dma_start(rt[:], Rh[l])
                nc.sync.dma_start(bt[:], Bh[l])
                R_sb.append(rt)
                Bm_sb.append(bt)

            def layernorm_block(zp, g_sl, b_sl, tag):
                st = stp.tile([128, 6], F32, tag="bst", name="bst")
                nc.vector.bn_stats(st[:], zp[:])
                mv = stp.tile([128, 2], F32, tag="bmv", name="bmv")
                nc.vector.bn_aggr(mv[:], st[:])
                srt = stp.tile([128, 1], F32, tag="srt", name="srt")
                nc.scalar.activation(srt[:], mv[:, 1:2], AF.Sqrt, bias=eps_t[:])
                rstd = stp.tile([128, 1], F32, tag="rsd", name="rsd")
                nc.vector.reciprocal(rstd[:], srt[:])
                mneg = stp.tile([128, 1], F32, tag="mng", name="mng")
                nc.vector.tensor_scalar(mneg[:], mv[:, 0:1], rstd[:], -1.0,
                                        ALU.mult, ALU.mult)
                zn = wk.tile([128, 256], BF16, tag="zn" + tag, name="zn" + tag)
                nc.vector.tensor_scalar(zn[:], zp[:], rstd[:], mneg[:],
                                        ALU.mult, ALU.add)
                xT = pp.tile([128, 2, 128], BF16, tag="ps", name="ps")
                nc.tensor.transpose(xT[:, 0, :], zn[:, 0:128], ident[:])
                nc.tensor.transpose(xT[:, 1, :], zn[:, 128:256], ident[:])
                x = wk.tile([128, 2, 128], BF16, tag="x" + tag, name="x" + tag)
                for k in range(2):
                    nc.scalar.activation(x[:, k, :], xT[:, k, :], AF.Relu,
                                         bias=b_sl[k][:], scale=g_sl[k][:])
                return x

            def relu_block(zp, rb_sl, tag):
                w = wk.tile([128, 256], BF16, tag="w" + tag, name="w" + tag)
                nc.vector.tensor_copy(w[:], zp[:])
                yT = pp.tile([128, 2, 128], BF16, tag="ps", name="ps")
                nc.tensor.transpose(yT[:, 0, :], w[:, 0:128], ident[:])
                nc.tensor.transpose(yT[:, 1, :], w[:, 128:256], ident[:])
                y = wk.tile([128, 2, 128], BF16, tag="y" + tag, name="y" + tag)
                for k in range(2):
                    nc.scalar.activation(y[:, k, :], yT[:, k, :], AF.Relu,
                                         bias=rb_sl[k][:])
                return y

            for l in range(L):
                cls_acc = accp.tile([128, NT, 10], F32, tag="clsa", name="clsa")
                tmp_acc = accp.tile([128, NT, 10], F32, tag="tmpa", name="tmpa")
                for t in range(NT):
                    z1 = pp.tile([128, 256], F32, tag="ps", name="ps")
                    nc.tensor.matmul(z1[:], hs_sb[l][0][:, ts(t, 128)],
                                     w_sb[l][0][0][:], start=True, stop=False)
                    nc.tensor.matmul(z1[:], hs_sb[l][1][:, ts(t, 128)],
                                     w_sb[l][0][1][:], start=False, stop=False)
                    nc.tensor.matmul(z1[:], ones[:], br_sb[l][0][:],
                                     start=False, stop=True)
                    x1 = layernorm_block(z1, sc_sb[l][0], sc_sb[l][1], "1")
                    z2 = pp.tile([128, 256], F32, tag="ps", name="ps")
                    nc.tensor.matmul(z2[:], x1[:, 0, :], w_sb[l][1][0][:],
                                     start=True, stop=False)
                    nc.tensor.matmul(z2[:], x1[:, 1, :], w_sb[l][1][1][:],
                                     start=False, stop=False)
                    nc.tensor.matmul(z2[:], ones[:], br_sb[l][1][:],
                                     start=False, stop=True)
                    x2 = layernorm_block(z2, sc_sb[l][2], sc_sb[l][3], "2")
                    cps = pp.tile([128, 10], F32, tag="ps", name="ps")
                    nc.tensor.matmul(cps[:], x2[:, 0, :], w3_sb[l][0][0][:],
                                     start=True, stop=False)
                    nc.tensor.matmul(cps[:], x2[:, 1, :], w3_sb[l][0][1][:],
                                     start=False, stop=False)
                    nc.tensor.matmul(cps[:], ones[:], br_sb[l][2][:, 0:10],
                                     start=False, stop=True)
                    nc.scalar.copy(cls_acc[:, t, :], cps[:])
                    r1 = pp.tile([128, 256], F32, tag="ps", name="ps")
                    nc.tensor.matmul(r1[:], hs_sb[l][0][:, ts(t, 128)],
                                     w_sb[l][2][0][:], start=True, stop=False)
                    nc.tensor.matmul(r1[:], hs_sb[l][1][:, ts(t, 128)],
                                     w_sb[l][2][1][:], start=False, stop=True)
                    y1 = relu_block(r1, sc_sb[l][4], "1")
                    r2 = pp.tile([128, 256], F32, tag="ps", name="ps")
                    nc.tensor.matmul(r2[:], y1[:, 0, :], w_sb[l][3][0][:],
                                     start=True, stop=False)
                    nc.tensor.matmul(r2[:], y1[:, 1, :], w_sb[l][3][1][:],
                                     start=False, stop=True)
                    y2 = relu_block(r2, sc_sb[l][5], "2")
                    tps = pp.tile([128, 10], F32, tag="ps", name="ps")
                    nc.tensor.matmul(tps[:], y2[:, 0, :], w3_sb[l][1][0][:],
                                     start=True, stop=False)
                    nc.tensor.matmul(tps[:], y2[:, 1, :], w3_sb[l][1][1][:],
                                     start=False, stop=False)
                    nc.tensor.matmul(tps[:], ones[:], br_sb[l][3][:, 0:10],
                                     start=False, stop=True)
                    nc.scalar.copy(tmp_acc[:, t, :], tps[:])

                e5 = wk.tile([128, NT, 5], F32, tag="e5", name="e5")
                nc.scalar.activation(e5[:], tmp_acc[:, :, 0:5], AF.Exp, bias=zer_t[:])
                num = wk.tile([128, NT, 5], F32, tag="num", name="num")
                nc.vector.tensor_tensor(num[:], e5[:], R_sb[l][:], ALU.mult)
                den = wk.tile([128, NT, 5], F32, tag="den", name="den")
                nc.vector.tensor_tensor(den[:], num[:], Bm_sb[l][:], ALU.add)
                rec = wk.tile([128, NT, 5], F32, tag="rec", name="rec")
                nc.vector.reciprocal(rec[:], den[:])
                crd = accp.tile([128, NT, 10], F32, tag="crd", name="crd")
                sg = wk.tile([128, NT, 5], F32, tag="sg", name="sg")
                nc.vector.tensor_tensor(sg[:], num[:], rec[:], ALU.mult)
                nc.vector.tensor_scalar(crd[:, :, 0:2], sg[:, :, 0:2],
                                        102.4, -51.2, ALU.mult, ALU.add)
                nc.vector.tensor_scalar(crd[:, :, 4:5], sg[:, :, 4:5],
                                        8.0, -5.0, ALU.mult, ALU.add)
                nc.vector.tensor_copy(crd[:, :, 2:4], tmp_acc[:, :, 2:4])
                nc.vector.tensor_copy(crd[:, :, 5:10], tmp_acc[:, :, 5:10])
                nc.sync.dma_start(o_cls[l].rearrange("t c p -> p t c"), cls_acc[:])
                nc.sync.dma_start(o_crd[l].rearrange("t c p -> p t c"), crd[:])

    nc.compile()
    return nc


def _prep_core_general(c, hs, init_reference, inter_references, W):
    bs = slice(c * BPC, (c + 1) * BPC)
    h = hs[:, :, bs, :]                                   # [L,Q,4,D]
    hsT = np.zeros((L, D, TP), np.float32)
    hsT[:, :, :T] = h.transpose(0, 3, 2, 1).reshape(L, D, BPC * Q)
    hsT = hsT.reshape(L, 2, 128, TP).astype(ml_dtypes.bfloat16)

    refs = np.concatenate([init_reference[None], inter_references[:L - 1]], 0)
    r = np.clip(refs[:, bs].reshape(L, T, 3), 0.0, 1.0)   # [L,3600,3]
    Ra = np.ones((L, TP, 5), np.float32)
    Rb = np.ones((L, TP, 5), np.float32)
    Ra[:, :T, 0:2] = np.maximum(r[:, :, 0:2], EPS)
    Ra[:, :T, 4] = np.maximum(r[:, :, 2], EPS)
    Rb[:, :T, 0:2] = np.maximum(1.0 - r[:, :, 0:2], EPS)
    Rb[:, :T, 4] = np.maximum(1.0 - r[:, :, 2], EPS)
    Rh = Ra.reshape(L, NT, 128, 5).transpose(0, 2, 1, 3).copy()
    Bh = Rb.reshape(L, NT, 128, 5).transpose(0, 2, 1, 3).copy()
    return dict(hsT=hsT, Rh=Rh, Bh=Bh, **W)


def _kernel_general(hs, init_reference, inter_references,
                    cls_w1, cls_b1, ln1_g, ln1_b, cls_w2, cls_b2, ln2_g, ln2_b,
                    cls_w3, cls_b3, reg_w1, reg_b1, reg_w2, reg_b2, reg_w3, reg_b3):
    wts = np.stack([cls_w1, cls_w2, reg_w1, reg_w2], 1).astype(ml_dtypes.bfloat16)
    wts = np.ascontiguousarray(wts.reshape(L, 4, 2, 128, 256))
    w3 = np.stack([cls_w3, reg_w3], 1).astype(ml_dtypes.bfloat16)
    w3 = np.ascontiguousarray(w3.reshape(L, 2, 2, 128, 10))
    brow = np.zeros((L, 4, 1, 256), np.float32)
    brow[:, 0, 0, :] = np.asarray(cls_b1).reshape(L, D)
    brow[:, 1, 0, :] = np.asarray(cls_b2).reshape(L, D)
    brow[:, 2, 0, :10] = np.asarray(cls_b3).reshape(L, 10)
    brow[:, 3, 0, :10] = np.asarray(reg_b3).reshape(L, 10)
    brow = brow.astype(ml_dtypes.bfloat16)
    scal = np.stack([np.asarray(x).reshape(L, D) for x in
                     (ln1_g, ln1_b, ln2_g, ln2_b, reg_b1, reg_b2)], 1)
    scal = np.ascontiguousarray(scal.reshape(L, 6, 256, 1).astype(np.float32))
    W = dict(wts=wts, w3p=w3, brow=brow, scal=scal)

    if "nc" not in _cache:
        _cache["nc"] = _build_general()
    nc = _cache["nc"]

    in_maps = [_prep_core_general(c, hs, init_reference, inter_references, W)
               for c in range(NCORES)]
    res = run_bass_kernel_spmd(nc, in_maps, core_ids=list(range(NCORES)),
                               trace=bool(os.environ.get("KTRACE")))
    _cache["last_result"] = res

    out = np.zeros((2, L, B, Q, 10), np.float32)
    for c in range(NCORES):
        for j, k in enumerate(("o_cls", "o_crd")):
            v = res.results[c][k]        # [L,NT,10,128]
            v = v.transpose(0, 1, 3, 2).reshape(L, TP, 10)[:, :T]
            out[j, :, c * BPC:(c + 1) * BPC] = v.reshape(L, BPC, Q, 10)
    return out
